# revision 23
# baseline (speedup 1.0000x reference)
"""Multi-head attention Trainium2 kernel (B=8, S=2048, EMB=768, H=4, Dh=192).

Strategy: data-parallel over batch — one batch element per NeuronCore, no
collectives. All layouts keep the PE contraction dim on partitions.

Design notes (v4):
  - Q^T/K^T stay SBUF-resident per head (no DRAM bounce): projection psums
    are evacuated straight into per-head lo[128,S]/hi[64,S] tiles (ACT for
    Q, fused +bq per-partition bias; DVE for K).
  - Exact math identities shave PE work: bk dropped (adds a per-row
    constant to scores, softmax-invariant); bv+bo folded into
    bo_eff = bo + bv @ Wo.T on the host (V proj has no bias row; bo_eff is
    added by DVE at the final evacuation, so out-proj has no ones row).
  - att@V runs "streamed": stationary = E[kpos, q-chunk], moving =
    V[kpos, dh+1] -> psum [q, dh+1]; 193-wide streams instead of padding
    Dh=192 up to 2x128-wide column blocks (25% fewer PE cycles than the
    column-stationary form). The appended ones column of V yields the
    softmax denominator Z in psum column 192 for free; 1/Z is applied as a
    per-partition ACT scale during evacuation (no broadcast needed).
  - The [q, dh] attention output is transposed back to feature-major via
    PE transpose (identity matmul, 128 cycles/tile) for the out-proj.
  - Per q-block interleave: scores/exp (h) -> transpose (h-1) -> attV (h),
    then the out-projection of the whole q-block; PE never crosses a phase
    barrier.
  - Aux tiles (identity, V ones-column, bo) are DMA'd at the very top:
    emitting them mid-phase raced ahead of their consumers on HW (first
    call returned slightly-wrong results; later calls read the previous
    call's identical data and looked fine).

  P1: V[s,dv] = xv.T @ WvT; Q^T[do,s] (+bq), K^T likewise (no bias)
  P2: per qb of 512, per head:
        scoresT[k,q] = KhT.T @ QhT ; E = exp(scale*scores)   (f16)
        out[q,dh]+Z  = E.T-stationary @ [Vh|1]  ; ACT evac scales by 1/Z
        oc^T[f,q]    = PE-transpose(out[q,dh])
      then out[s,e] = OcatT.T @ WoT for that qb (+bo_eff at evac, f16 out)
"""

import os
import sys

sys.path.insert(0, "/opt/trn_rl_repo")
# rotate PSUM/SBUF pool slots instead of stack-reusing just-released ones:
# avoids WAR stalls on the PE at phase transitions
os.environ.setdefault("TILE_POOL_ALLOC_MODE", "queue")

import numpy as np

import concourse.bass as bass  # noqa: F401  (import keeps bass registered)
import concourse.mybir as mybir
import concourse.tile as tile
from concourse import bacc

B, S, EMB, HEADS = 8, 2048, 768, 4
DH = EMB // HEADS  # 192
NCORES = 8
P = 128
DI_TILES = EMB // P  # 6
S_TILES = S // P  # 16
QBLK = 512
N_QBLK = S // QBLK  # 4
EBLK = 384
SCALE = 1.0 / float(np.sqrt(DH))
VW = DH + 1  # 193 cols per head in the V tile (192 dh + ones col for Z)

F32 = mybir.dt.float32
F16 = mybir.dt.float16


def _qk_evac_segments(j):
    """Rows of proj psum block j -> per-head (lo/hi, row) segments."""
    segs = []
    p = 0
    while p < P:
        r = j * P + p
        h, d = divmod(r, DH)
        if d < P:
            run = min(P - p, P - d)
            segs.append((p, run, h, "lo", d))
        else:
            run = min(P - p, DH - d)
            segs.append((p, run, h, "hi", d - P))
        p += run
    return segs


def _build_nc(reps=1, phases=3):
    nc = bacc.Bacc("TRN2", target_bir_lowering=False, debug=False,
                   num_devices=NCORES, enable_partition_id=False)

    # packed inputs (fewer parameters = less per-call dispatch overhead
    # over the axon tunnel):
    #   x16 rows: [0:768) q^T | [768:1536) k^T | [1536:2304) v^T
    #   w16 rows: [0:768) WqT | [768:1536) WkT | [1536:2304) WvT |
    #             [2304:3072) WoT | [3072:3200) ones | [3200:3328) identity
    #   aux32 cols: [0:768) bo_eff broadcast | [768:774) bq column-tiles
    NW = 4 * EMB + 3 * P + 1  # 3457 rows of the weights block
    NFLAT = 3 * EMB * S + NW * EMB
    flat = nc.declare_dram_parameter("flat", [1, NFLAT], F16, isOutput=False)
    x16 = flat[0:1, 0:3 * EMB * S].rearrange("o (r c) -> (o r) c", c=S)
    w16 = flat[0:1, 3 * EMB * S:NFLAT].rearrange("o (r c) -> (o r) c", c=EMB)
    xq, xk, xv = x16[0:EMB, :], x16[EMB:2 * EMB, :], x16[2 * EMB:3 * EMB, :]
    wq, wk = w16[0:EMB, :], w16[EMB:2 * EMB, :]
    wv, wo = w16[2 * EMB:3 * EMB, :], w16[3 * EMB:4 * EMB, :]
    ones16 = w16[4 * EMB:4 * EMB + P, :]
    ident = w16[4 * EMB + P:4 * EMB + 2 * P, :]
    bo_bc = w16[4 * EMB + 2 * P:4 * EMB + 3 * P, :]
    bqrow = w16[4 * EMB + 3 * P:4 * EMB + 3 * P + 1, :]
    out = nc.declare_dram_parameter("out", [S, EMB], F16, isOutput=True)

    with tile.TileContext(nc) as tc:
        with tc.tile_pool(name="res", bufs=1) as res:
            # SBUF residents
            v_sb = res.tile([P, S_TILES, HEADS * VW], F16, name="v_sb")
            kh = [res.tile([P, S], F16, name=f"kh{h}", tag=f"kh{h}")
                  for h in range(HEADS)]
            khh = [res.tile([DH - P, S], F16, name=f"khh{h}", tag=f"khh{h}")
                   for h in range(HEADS)]
            qh = [res.tile([P, S], F16, name=f"qh{h}", tag=f"qh{h}")
                  for h in range(HEADS)]
            qhh = [res.tile([DH - P, S], F16, name=f"qhh{h}", tag=f"qhh{h}")
                   for h in range(HEADS)]
            oc_sb = [res.tile([P, S], F16, name=f"oc{j}", tag=f"oc{j}")
                     for j in range(DI_TILES)]
            id_sb = res.tile([P, P], F16, name="id_sb")
            bo16_sb = res.tile([P, EMB], F16, name="bo16_sb")
            bo_sb = res.tile([P, EMB], F32, name="bo_sb")
            ones_sb = res.tile([1, QBLK], F16, name="ones_sb")
            bqv_sb = res.tile([1, EMB], F16, name="bqv_sb")
            nc.sync.dma_start(out=id_sb, in_=ident[:, 0:P])
            nc.sync.dma_start(
                out=v_sb.rearrange(
                    "p t (h c) -> p t h c", c=VW)[:, :, :, DH],
                in_=ones16[:, 0:S_TILES * HEADS].rearrange(
                    "p (t h) -> p t h", h=HEADS))
            nc.sync.dma_start(out=bo16_sb, in_=bo_bc[:, :])
            nc.sync.dma_start(out=ones_sb, in_=ones16[0:1, 0:QBLK])
            nc.sync.dma_start(out=bqv_sb, in_=bqrow[0:1, :])
            nc.vector.tensor_copy(bo_sb, bo16_sb)

            for rep in range(reps):
                # Phase 1b weights pool opens early so its DMAs can be
                # emitted right after the first xv block (FIFO DMA queue:
                # weights land long before the K/Q matmuls need them).
                wp = tc.alloc_tile_pool(name=f"w1_{rep}", bufs=1)
                wk_t = [wp.tile([P, EMB], F16, name=f"wk{i}", tag=f"wk{i}")
                        for i in range(DI_TILES)]
                wq_t = [wp.tile([P, EMB], F16, name=f"wq{i}", tag=f"wq{i}")
                        for i in range(DI_TILES)]

                # ---------------- Phase 1a: V projection -------------------
                with tc.tile_pool(name=f"wv1_{rep}", bufs=1) as wvp, \
                     tc.tile_pool(name=f"xv1_{rep}", bufs=2) as xvp, \
                     tc.tile_pool(name=f"psv_{rep}", bufs=2, space="PSUM") as psv:
                    wv_sb = wvp.tile([P, DI_TILES, EMB], F16, name="wv_sb")
                    for i in range(DI_TILES):
                        nc.sync.dma_start(out=wv_sb[:, i, :],
                                          in_=wv[i * P:(i + 1) * P, :])

                    for sb in range(N_QBLK):
                        xv_sb = xvp.tile([P, DI_TILES, QBLK], F16, name="xv_sb",
                                         tag="xv")
                        for i in range(DI_TILES):
                            nc.sync.dma_start(
                                out=xv_sb[:, i, :],
                                in_=xv[i * P:(i + 1) * P,
                                       sb * QBLK:(sb + 1) * QBLK])
                        if sb == 0:
                            for i in range(DI_TILES):
                                nc.sync.dma_start(
                                    out=wk_t[i], in_=wk[i * P:(i + 1) * P, :])
                        elif sb == 1:
                            for i in range(DI_TILES):
                                nc.sync.dma_start(
                                    out=wq_t[i], in_=wq[i * P:(i + 1) * P, :])
                        for sti in range(QBLK // P):
                            st = sb * (QBLK // P) + sti
                            pcols = slice(sti * P, (sti + 1) * P)
                            for blk in range(2):  # heads {0,1} then {2,3}
                                ps = psv.tile([P, EBLK], F32, name="vproj",
                                              tag="vproj")
                                for di in range(DI_TILES):
                                    nc.tensor.matmul(
                                        ps, xv_sb[:, di, pcols],
                                        wv_sb[:, di,
                                              blk * EBLK:(blk + 1) * EBLK],
                                        start=(di == 0),
                                        stop=(di == DI_TILES - 1))
                                # evacuate into v_sb with per-head gap (ones col)
                                dst = v_sb[:, st, blk * 2 * VW:(blk * 2 + 2) * VW]
                                dst = dst.rearrange("p (h c) -> p h c",
                                                    c=VW)[:, :, 0:DH]
                                with nc.allow_low_precision(
                                        reason="f16 V storage by design"):
                                    nc.vector.tensor_copy(
                                        dst,
                                        ps.rearrange("p (h c) -> p h c", c=DH))

                # ---------------- Phase 1b: K and Q projections ------------
                with tc.tile_pool(name=f"x1_{rep}", bufs=2) as xp, \
                     tc.tile_pool(name=f"ps1_{rep}", bufs=3, space="PSUM") as psp:
                    for (xin, wt, is_q, lo_t, hi_t) in (
                            (xk, wk_t, False, kh, khh),
                            (xq, wq_t, True, qh, qhh)):
                        for sb in range(N_QBLK):
                            xs = xp.tile([P, DI_TILES, QBLK], F16, name="xs",
                                         tag="xs")
                            for i in range(DI_TILES):
                                nc.sync.dma_start(
                                    out=xs[:, i, :],
                                    in_=xin[i * P:(i + 1) * P,
                                            sb * QBLK:(sb + 1) * QBLK])
                            scols = slice(sb * QBLK, (sb + 1) * QBLK)
                            for do in range(DI_TILES):
                                ps = psp.tile([P, QBLK], F32, name="proj",
                                              tag="proj")
                                for di in range(DI_TILES):
                                    nc.tensor.matmul(
                                        ps, wt[di][:, do * P:(do + 1) * P],
                                        xs[:, di, :],
                                        start=(di == 0),
                                        stop=(di == DI_TILES - 1 and not is_q))
                                if is_q:
                                    # +bq via ones-row rank-1 matmul
                                    nc.tensor.matmul(
                                        ps, bqv_sb[0:1, do * P:(do + 1) * P],
                                        ones_sb[0:1, :],
                                        start=False, stop=True)
                                for (p0, run, h, kind, row) in \
                                        _qk_evac_segments(do):
                                    dst_t = (lo_t if kind == "lo" else hi_t)[h]
                                    dst = dst_t[row:row + run, scols]
                                    with nc.allow_low_precision(
                                            reason="f16 Q/K storage by design"):
                                        nc.vector.tensor_copy(
                                            dst, ps[p0:p0 + run, :])

                wp.release()

                if phases >= 2:
                    # -------- Phase 2+3: attention + output proj, per qb ----
                    with tc.tile_pool(name=f"wo_{rep}", bufs=1) as wp3, \
                         tc.tile_pool(name=f"ee_{rep}", bufs=2) as eep, \
                         tc.tile_pool(name=f"zz_{rep}", bufs=4) as zzp, \
                         tc.tile_pool(name=f"ocq_{rep}", bufs=8) as ocqp, \
                         tc.tile_pool(name=f"ev3_{rep}", bufs=3) as evp3, \
                         tc.tile_pool(name=f"pse_{rep}", bufs=2, space="PSUM") as pse, \
                         tc.tile_pool(name=f"pso_{rep}", bufs=2, space="PSUM") as pso, \
                         tc.tile_pool(name=f"pst_{rep}", bufs=2, space="PSUM") as pst, \
                         tc.tile_pool(name=f"ps3_{rep}", bufs=2, space="PSUM") as psp3:
                        wo_t = [wp3.tile([P, EMB], F16, name=f"wo{i}",
                                         tag=f"wo{i}") for i in range(DI_TILES)]
                        for i in range(DI_TILES):
                            nc.sync.dma_start(out=wo_t[i],
                                              in_=wo[i * P:(i + 1) * P, :])

                        NQC = QBLK // P  # 4 q-chunks of 128 per q-block

                        def _transpose_head(h, qb, ocq_tiles):
                            # ocq [128 q, DH] -> oc_sb[j] rows via PE
                            # transpose (f16) + DVE psum evacuation
                            for qc in range(NQC):
                                for f0, flen in ((0, P), (P, DH - P)):
                                    ps_t = pst.tile([P, P], F16, name="pst",
                                                    tag="pst")
                                    nc.tensor.transpose(
                                        ps_t[0:flen, :],
                                        ocq_tiles[qc][:, f0:f0 + flen],
                                        id_sb)
                                    # rows = global features h*DH+f0 ...
                                    r0 = h * DH + f0
                                    a = 0
                                    while a < flen:
                                        j, p0 = divmod(r0 + a, P)
                                        run = min(flen - a, P - p0)
                                        nc.vector.tensor_copy(
                                            oc_sb[j][p0:p0 + run,
                                                     qb * QBLK + qc * P:
                                                     qb * QBLK + (qc + 1) * P],
                                            ps_t[a:a + run, :])
                                        a += run

                        for qb in range(N_QBLK):
                            qcols = slice(qb * QBLK, (qb + 1) * QBLK)
                            pending = None  # (h, ocq_tiles)
                            for h in range(HEADS):
                                # scores + exp for head h
                                e_all = eep.tile([P, S_TILES, QBLK], F16,
                                                 name="E", tag="E")
                                for kt in range(S_TILES):
                                    kc = slice(kt * P, (kt + 1) * P)
                                    ps_e = pse.tile([P, QBLK], F32, name="pse",
                                                    tag="pse")
                                    nc.tensor.matmul(ps_e, kh[h][:, kc],
                                                     qh[h][:, qcols],
                                                     start=True, stop=False)
                                    nc.tensor.matmul(ps_e, khh[h][:, kc],
                                                     qhh[h][:, qcols],
                                                     start=False, stop=True)
                                    nc.scalar.activation(
                                        e_all[:, kt, :], ps_e,
                                        mybir.ActivationFunctionType.Exp,
                                        bias=0.0, scale=SCALE)

                                # transpose previous head while PE is busy
                                if pending is not None:
                                    _transpose_head(pending[0], qb, pending[1])
                                    pending = None

                                # att @ V, streamed: psum [q, dh+1], V moving
                                vh = v_sb.rearrange("p t (h c) -> p t h c",
                                                    c=VW)
                                ocq_tiles = []
                                for qc in range(NQC):
                                    ps_av = pso.tile([P, VW], F32, name="av",
                                                     tag="av")
                                    for kt in range(S_TILES):
                                        nc.tensor.matmul(
                                            ps_av,
                                            e_all[:, kt,
                                                  qc * P:(qc + 1) * P],
                                            vh[:, kt, h, :],
                                            start=(kt == 0),
                                            stop=(kt == S_TILES - 1))
                                    rz = zzp.tile([P, 1], F32, name="rz",
                                                  tag="rz")
                                    nc.vector.reciprocal(
                                        rz, ps_av[:, DH:DH + 1])
                                    ocq = ocqp.tile([P, DH], F16, name="ocq",
                                                    tag="ocq")
                                    with nc.allow_low_precision(
                                            reason="softmax normalize, f16"):
                                        nc.scalar.activation(
                                            ocq, ps_av[:, 0:DH],
                                            mybir.ActivationFunctionType.Copy,
                                            bias=0.0, scale=rz)
                                    ocq_tiles.append(ocq)
                                pending = (h, ocq_tiles)

                            _transpose_head(pending[0], qb, pending[1])

                            if phases >= 3:
                                # output projection for this qb
                                for sti in range(QBLK // P):
                                    st = qb * (QBLK // P) + sti
                                    scols = slice(st * P, (st + 1) * P)
                                    for eb in range(2):
                                        ecols = slice(eb * EBLK, (eb + 1) * EBLK)
                                        ps = psp3.tile([P, EBLK], F32,
                                                       name="fin", tag="fin")
                                        for j in range(DI_TILES):
                                            nc.tensor.matmul(
                                                ps, oc_sb[j][:, scols],
                                                wo_t[j][:, ecols],
                                                start=(j == 0),
                                                stop=(j == DI_TILES - 1))
                                        fin = evp3.tile([P, EBLK], F16,
                                                        name="fin_sb",
                                                        tag="fin_sb")
                                        with nc.allow_low_precision(
                                                reason="f16 output by design"):
                                            nc.vector.tensor_add(
                                                fin, ps, bo_sb[:, ecols])
                                        nc.gpsimd.dma_start(
                                            out=out[scols, ecols], in_=fin)

    nc.compile()
    return nc


_CACHE = {}


def _get_runner(reps=1, phases=3):
    """Build nc once and a reusable jitted SPMD callable (no recompiles)."""
    key = f"runner{reps}_{phases}"
    if key in _CACHE:
        return _CACHE[key]

    import jax
    import numpy as _np
    from jax.sharding import Mesh, PartitionSpec
    from jax.experimental.shard_map import shard_map
    from concourse import bass2jax
    from concourse.bass2jax import _bass_exec_p, install_neuronx_cc_hook

    nc = _build_nc(reps, phases)
    install_neuronx_cc_hook()

    partition_name = (nc.partition_id_tensor.name
                      if nc.partition_id_tensor else None)
    in_names, out_names, out_avals, zero_outs = [], [], [], []
    for alloc in nc.m.functions[0].allocations:
        if not isinstance(alloc, mybir.MemoryLocationSet):
            continue
        name = alloc.memorylocations[0].name
        if alloc.kind == "ExternalInput":
            if name != partition_name:
                in_names.append(name)
        elif alloc.kind == "ExternalOutput":
            shape = list(alloc.tensor_shape)
            npdt = mybir.dt.np(alloc.dtype)
            out_avals.append(jax.core.ShapedArray(shape, npdt))
            out_names.append(name)
            zero_outs.append(_np.zeros(shape, npdt))
    n_params = len(in_names)
    n_outs = len(out_names)
    in_names = in_names + out_names
    if partition_name is not None:
        in_names.append(partition_name)

    def _body(*args):
        operands = list(args)
        if partition_name is not None:
            operands.append(bass2jax.partition_id_tensor())
        outs = _bass_exec_p.bind(
            *operands,
            out_avals=tuple(out_avals),
            in_names=tuple(in_names),
            out_names=tuple(out_names),
            lowering_input_output_aliases=(),
            sim_require_finite=True,
            sim_require_nnan=True,
            nc=nc,
        )
        return tuple(outs)

    devices = jax.devices()[:NCORES]
    mesh = Mesh(_np.asarray(devices), ("core",))
    in_specs = (PartitionSpec("core"),) * (n_params + n_outs)
    out_specs = (PartitionSpec("core"),) * n_outs
    sharded = jax.jit(
        shard_map(_body, mesh=mesh, in_specs=in_specs, out_specs=out_specs,
                  check_rep=False),
        keep_unused=True,
    )
    concat_zeros = [
        _np.zeros((NCORES * z.shape[0], *z.shape[1:]), z.dtype)
        for z in zero_outs
    ]

    runner = {
        "nc": nc, "sharded": sharded, "in_names": in_names,
        "n_params": n_params, "out_names": out_names,
        "out_avals": out_avals, "concat_zeros": concat_zeros,
        "mesh": mesh,
    }
    _CACHE[key] = runner
    return runner


def run_spmd(in_maps):
    """Run the compiled SPMD program; in_maps is a list of NCORES dicts."""
    import numpy as _np
    r = _get_runner()
    per_core = [[_np.asarray(m[name]) for name in r["in_names"][:r["n_params"]]]
                for m in in_maps]
    concat_in = [
        _np.concatenate([per_core[c][i] for c in range(NCORES)], axis=0)
        for i in range(r["n_params"])
    ]
    out_arrs = r["sharded"](*concat_in, *r["concat_zeros"])
    return [
        {name: _np.asarray(out_arrs[i]).reshape(NCORES, *r["out_avals"][i].shape)[c]
         for i, name in enumerate(r["out_names"])}
        for c in range(NCORES)
    ]


def _prep_in_maps(q, k, v, Wq, bq, Wk, bk, Wv, bv, Wo, bo):
    q = np.asarray(q, dtype=np.float32)
    k = np.asarray(k, dtype=np.float32)
    v = np.asarray(v, dtype=np.float32)
    Wo32 = np.asarray(Wo, np.float32)
    bo_eff = np.asarray(bo, np.float32) + np.asarray(bv, np.float32) @ Wo32.T
    NW = 4 * EMB + 3 * P + 1
    w16 = np.empty((NW, EMB), np.float16)
    w16[0:EMB] = np.asarray(Wq, np.float32).T.astype(np.float16)
    w16[EMB:2 * EMB] = np.asarray(Wk, np.float32).T.astype(np.float16)
    w16[2 * EMB:3 * EMB] = np.asarray(Wv, np.float32).T.astype(np.float16)
    w16[3 * EMB:4 * EMB] = Wo32.T.astype(np.float16)
    w16[4 * EMB:4 * EMB + P] = np.float16(1.0)
    w16[4 * EMB + P:4 * EMB + 2 * P] = 0
    w16[4 * EMB + P:4 * EMB + 2 * P, 0:P] = np.eye(P, dtype=np.float16)
    w16[4 * EMB + 2 * P:4 * EMB + 3 * P] = bo_eff[None, :].astype(np.float16)
    w16[4 * EMB + 3 * P] = np.asarray(bq, np.float32).astype(np.float16)
    wflat = w16.reshape(-1)
    in_maps = []
    for b in range(NCORES):
        x16 = np.empty((3 * EMB, S), np.float16)
        x16[0:EMB] = q[b].T.astype(np.float16)
        x16[EMB:2 * EMB] = k[b].T.astype(np.float16)
        x16[2 * EMB:3 * EMB] = v[b].T.astype(np.float16)
        flat = np.concatenate([x16.reshape(-1), wflat])[None, :]
        in_maps.append({"flat": flat})
    return in_maps


def kernel(q, k, v, Wq, bq, Wk, bk, Wv, bv, Wo, bo):
    in_maps = _prep_in_maps(q, k, v, Wq, bq, Wk, bk, Wv, bv, Wo, bo)
    results = run_spmd(in_maps)
    out = np.stack([results[b]["out"] for b in range(NCORES)], axis=0)
    return out.astype(np.float32)
